# revision 1
# baseline (speedup 1.0000x reference)
"""Trainium2 Bass kernel for CAM (channel attention module).

reference:
    q = k = x2.reshape(B, C, N); v = x.reshape(B, C, N)   # B=8, C=512, N=4096
    energy = q @ q^T                # [B, C, C]
    att = softmax(energy, axis=-1)
    out = att @ v
    y = gamma * out + x

Sharding: data-parallel over batch, one batch element per NeuronCore (8 cores).
Each core computes its own [C, N] slice end to end; no collectives.

Per-core dataflow (C=512, N=4096, P=128):
  1. stream x2 in 512-wide column chunks (one fused SWDGE cast-DMA
     f32 -> bf16 per chunk covering all 4 c-tiles; the first chunk is two
     256-wide halves so the PE starts sooner). PE-transpose 128x128 blocks
     -> qT bf16 [n-part, c-free]; MM1 for chunk i-1 is emitted right after
     chunk i's transposes, so the PE sees one continuous stream.
  2. MM1 (bf16): E [C, C] in PSUM f32, symmetric: each row-tile
     accumulates one strip [m*P:] (diag included; one PSUM group per
     bank); lower blocks are PE transposes of the mirrors. Final-chunk
     strips are emitted m=3 -> m=0 so the narrow strips stop first.
  3. softmax shift = Gram diagonal ||q_c||^2 (valid shift by symmetry;
     softmax is shift-invariant): partial sums of squares accumulate on
     ACT (Square+accum) and DVE (mult+reduce) during the otherwise-idle
     load phase, tree-add on gpsimd, then one rank-1 PE matmul broadcasts
     the negated row to [128, C] well before the strips stop.
  4. attT_m = exp(E_m - shift[free]) in two pieces per row-tile: the strip
     region [m*P:] fires the moment strip m stops, the mirror region
     [:m*P] once the mirrors land. gpsimd mirrors each piece into fp8.
  5. MM2: per output tile, the diagonal block and its neighbour stay bf16;
     the two far k-blocks (att ~ exp(-large)) run as one fp8e4 DoubleRow
     matmul at half the PE rows. m=3 goes first, term-outer across KOUT
     chunks, because its att columns avoid the mirror region entirely --
     MM2 starts right at the first exp and the PE p-state never drops.
     Softmax row sums ride along as PE column-sums of attT (symmetry).
  6. y = (out * gamma/s_c) + x_bf16 via ACT-scale (per-partition gs AP)
     + bf16 DVE add; the PSUM bank frees at the ACT copy so MM2 stays
     PE-paced. Stored as bf16 (host upcasts; ~3e-3 rel err, 2e-2 gate).

Known hardware pitfalls encoded here: GPSIMD must not touch PSUM;
tensor_tensor_reduce crashes the exec unit (use mult + tensor_reduce);
only gpsimd DMAs can cast; PSUM allows one open accumulation group per
bank; list-scheduler hoists ready DMAs past program order (pin with
sliver dependencies).
"""

import numpy as np

import concourse.bass as bass
import concourse.mybir as mybir
from concourse import bacc
from concourse.tile import TileContext
from concourse.masks import make_identity

P = 128
C = 512
N = 4096
B = 8
IC = C // P          # 4 c-tiles
JN = N // P          # 32 n-tiles
F32 = mybir.dt.float32
BF16 = mybir.dt.bfloat16
FP8 = mybir.dt.float8e4

QCHUNK = 512         # x2 load chunk width (free dim)
# first chunk split in two so the PE pipeline starts ~2us earlier
CHUNK_W = [QCHUNK // 2, QCHUNK // 2] + [QCHUNK] * (N // QCHUNK - 1)
CHUNK_J0 = [sum(CHUNK_W[:i]) // P for i in range(len(CHUNK_W))]
NCB = len(CHUNK_W)   # chunks
YCHUNK = 1024        # y store chunk width
NH = N // YCHUNK     # y chunks per row-tile
KOUT = 2             # m=3 chunks run term-outer (po has POB bufs)
POB = 3              # MM2 PSUM output buffers ([P,1024] f32 = 2 banks each)


def _emit_core(nc, tc, x, x2, gamma, y):
    with (
            tc.tile_pool(name="small", bufs=1) as small,
            tc.tile_pool(name="vpool", bufs=2) as pool_v,
            tc.tile_pool(name="att", bufs=2) as pool_att,
            tc.tile_pool(name="scr", bufs=3) as pool_scr,
            tc.tile_pool(name="ypool", bufs=4) as pool_y,
            tc.tile_pool(name="qn_", bufs=NCB - 2) as pool_qn,
            tc.tile_pool(name="qt_", bufs=1) as pool_qt,
        ):
            # chunk 0's cast-DMA desc-gen goes first on the Pool engine;
            # the transpose identity (only constant the x2 pipeline needs)
            # builds while that DMA is in flight
            x2v = x2.rearrange("(i p) n -> p i n", p=P)
            qn_chunks = []

            def emit_qn_dma(cb):
                # one fused cast-DMA per chunk for all 4 c-tiles
                # (SWDGE fixed desc-gen cost is per-instruction)
                w0, wd = CHUNK_J0[cb] * P, CHUNK_W[cb]
                qn = pool_qn.tile([P, IC, QCHUNK], BF16, tag="qn")
                nc.gpsimd.dma_start(qn[:, :, :wd], x2v[:, :, w0:w0 + wd])
                qn_chunks.append(qn)

            emit_qn_dma(0)
            ident_bf = small.tile([P, P], BF16, tag="ident_bf")
            make_identity(nc, ident_bf)
            for cb in range(1, NCB):
                emit_qn_dma(cb)

            # --- remaining constants / tiny tensors ---
            ident_f32 = small.tile([P, P], F32, tag="ident_f32")
            make_identity(nc, ident_f32)
            ones_row = small.tile([1, P], F32, tag="ones_row")
            nc.any.memset(ones_row, 1.0)
            neg_ones = small.tile([1, P], F32, tag="neg_ones")
            nc.any.memset(neg_ones, -1.0)
            ones_col = small.tile([P, 1], BF16, tag="ones_col")
            nc.any.memset(ones_col, 1.0)
            # preload the Exp table off the critical path
            warm_act = small.tile([1, P], BF16, tag="warm_act")
            nc.scalar.activation(
                warm_act, ones_row, mybir.ActivationFunctionType.Exp
            )
            g_sb = small.tile([1, 1], F32, tag="g_sb")
            nc.scalar.dma_start(g_sb, gamma[:, :])
            gvec = small.tile([P, 1], F32, tag="gvec")
            with tc.tile_pool(name="pg", bufs=1, space="PSUM") as pg:
                gp = pg.tile([P, 1], F32, tag="gp")
                # gvec[p] = gamma for all p  (rank-1 broadcast via PE)
                nc.tensor.matmul(gp, lhsT=ones_row, rhs=g_sb, start=True, stop=True)
                nc.vector.tensor_copy(gvec, gp)

            xb_tiles = []
            xb8 = pool_v.tile([P, IC, N], FP8, tag="xb8")
            qt = pool_qt.tile([P, JN, P * IC], BF16, tag="qt")  # [128,32,512]

            # pb holds the tiny PSUM tensors whose lifetime crosses the
            # MM1 -> MM2 boundary (2 banks, rotating: shift row / shift
            # broadcast / row-sum row / row-sum columns)
            import contextlib
            with contextlib.nullcontext():
                with (
                    tc.tile_pool(name="pt", bufs=2, space="PSUM") as pt,
                    tc.tile_pool(name="pe_", bufs=4, space="PSUM") as pe_,
                ):
                    # E accumulators live across the whole streamed MM1
                    e_tiles = [pe_.tile([P, C], F32, tag="E", name=f"E{m}")
                               for m in range(IC)]

                    # E is symmetric: each row-tile accumulates one strip
                    # [m*P:] (diag included -- a PSUM bank allows only one
                    # open accumulation group); lower blocks are PE
                    # transposes of the mirrors, filled in as strips stop.
                    def emit_strip(cb, m, stop):
                        jpc = CHUNK_W[cb] // P
                        for jj in range(jpc):
                            j = CHUNK_J0[cb] + jj
                            nc.tensor.matmul(
                                e_tiles[m][:, m * P:],
                                lhsT=qt[:, j, m * P:(m + 1) * P],
                                rhs=qt[:, j, m * P:],
                                start=(j == 0),
                                stop=(stop and jj == jpc - 1),
                            )

                    def emit_mm1(cb):
                        for m in range(IC):
                            emit_strip(cb, m, stop=False)

                    ssq_parts = [[None] * NCB for _ in range(IC)]
                    negmb = small.tile([P, C], F32, tag="negmb")

                    def emit_x_loads():
                        # v (= x) cast-loads queue behind x2 on the same
                        # SWDGE FIFO (desc-gen before the gpsimd tree-adds);
                        # bf16 serves the diag/single MM2 terms and the
                        # residual add, fp8 the DoubleRow pair terms. Each
                        # DMA overwrites a sliver copied from a late x2 chunk
                        # so the scheduler cannot hoist it ahead of the
                        # stream.
                        for k in range(IC):
                            xb = pool_v.tile([P, N], BF16, tag=f"xb{k}",
                                             name=f"xb{k}")
                            nc.gpsimd.tensor_copy(
                                xb[:, :1], qn_chunks[NCB - 2][:, 0, :1]
                            )
                            nc.gpsimd.dma_start(xb, x[k * P:(k + 1) * P, :])
                            xb_tiles.append(xb)
                        xv = x.rearrange("(i p) n -> p i n", p=P)
                        nc.gpsimd.tensor_copy(
                            xb8[:, 0, :1], qn_chunks[NCB - 2][:, 0, :1]
                        )
                        nc.gpsimd.dma_start(
                            xb8[:, :, :N // 2], xv[:, :, :N // 2]
                        )
                        nc.gpsimd.dma_start(
                            xb8[:, :, N // 2:], xv[:, :, N // 2:]
                        )

                    def emit_shift_chain():
                        # ssq_i = ||q_c||^2 per c-tile: tree-add the square
                        # partials on gpsimd (idle after desc-gen; SBUF-only),
                        # then shift as a row [1, C], broadcast (negated, via
                        # the minus-ones lhsT) to [128, C] on PE. Emitted
                        # inside the load loop so the PE hits the transposes
                        # when their inputs are already long ready, well
                        # before MM1's final strips stop. The PSUM pool is
                        # scoped to the chain so its banks free immediately.
                        ssq = [None] * IC
                        for i in range(IC - 1, -1, -1):
                            acc = small.tile([P, 1], F32, tag=f"ssqa{i}",
                                             name=f"ssqa{i}")
                            nc.gpsimd.tensor_tensor(
                                acc, ssq_parts[i][0], ssq_parts[i][1],
                                mybir.AluOpType.add,
                            )
                            for cc in range(2, NCB):
                                nc.gpsimd.tensor_tensor(
                                    acc, acc, ssq_parts[i][cc],
                                    mybir.AluOpType.add,
                                )
                            ssq[i] = acc
                        # built per 128-column piece, m=3 first, so each
                        # piece's exp chain fires as soon as its strip stops
                        # instead of waiting for the full-row broadcast
                        with tc.tile_pool(name="pbx", bufs=1,
                                          space="PSUM") as pbx:
                            mrow_p = pbx.tile([1, C], F32, tag="rowbuf")
                            mrow_sb = small.tile([1, C], F32, tag="mrow_sb")
                            negmb_p = pbx.tile([P, C], F32, tag="bcbuf")
                            for m in range(IC - 1, -1, -1):
                                c0, c1 = m * P, (m + 1) * P
                                nc.tensor.transpose(
                                    mrow_p[:, c0:c1], ssq[m], ident_f32,
                                )
                                nc.vector.tensor_copy(
                                    mrow_sb[:, c0:c1], mrow_p[:, c0:c1]
                                )
                                nc.tensor.matmul(
                                    negmb_p[:, c0:c1], lhsT=neg_ones,
                                    rhs=mrow_sb[:, c0:c1],
                                    start=True, stop=True,
                                )
                                nc.scalar.copy(
                                    negmb[:, c0:c1], negmb_p[:, c0:c1]
                                )

                    for cb in range(NCB):
                        qn = qn_chunks[cb]
                        for jj in range(CHUNK_W[cb] // P):
                            j = CHUNK_J0[cb] + jj
                            ps = pt.tile([P, P * IC], BF16, tag="ps")
                            for i in range(IC):
                                nc.tensor.transpose(
                                    ps[:, i * P:(i + 1) * P],
                                    qn[:, i, jj * P:(jj + 1) * P],
                                    ident_bf,
                                )
                            nc.vector.tensor_copy(out=qt[:, j, :], in_=ps)
                        # partial sum-of-squares of q: the softmax shift is
                        # the Gram diagonal ||q_c||^2 (instead of a row max),
                        # computed on otherwise-idle ACT/DVE during the load
                        # phase so the shift broadcast is ready long before
                        # MM1 finishes
                        for i in range(IC):
                            pp = small.tile([P, 1], F32, tag=f"ssq{i}_{cb}",
                                            name=f"ssq{i}_{cb}")
                            sq = pool_scr.tile([P, QCHUNK], BF16, tag="sq",
                                               name="sq")
                            wd = CHUNK_W[cb]
                            if (cb * IC + i) % 3 != 1:
                                nc.scalar.activation(
                                    sq[:, :wd], qn[:, i, :wd],
                                    mybir.ActivationFunctionType.Square,
                                    accum_out=pp,
                                )
                            else:
                                # (tensor_tensor_reduce crashes the exec
                                # unit on TRN2 hardware; use mult + reduce)
                                nc.vector.tensor_tensor(
                                    sq[:, :wd], qn[:, i, :wd], qn[:, i, :wd],
                                    mybir.AluOpType.mult,
                                )
                                nc.vector.tensor_reduce(
                                    pp, sq[:, :wd], mybir.AxisListType.X,
                                    mybir.AluOpType.add,
                                )
                            ssq_parts[i][cb] = pp
                        if cb == NCB - 1:
                            emit_x_loads()
                            emit_shift_chain()
                        if cb > 0:
                            emit_mm1(cb - 1)

                    # --- final chunk: staggered strips -> mirrors -> exp ---
                    cb = NCB - 1

                    def emit_mirrors(n):
                        # lower blocks of rows m > n from strip n's mirrors:
                        # E_m[:, n] = E_n[:, m]^T (copies on ACT: it may read
                        # PSUM, keeping DVE free for the tmp adds)
                        for m in range(n + 1, IC):
                            eb = pool_scr.tile([P, P], F32, tag="eb", name="eb")
                            ceng = nc.scalar if (m + n) % 2 == 0 else nc.vector
                            (ceng.copy if ceng is nc.scalar
                             else ceng.tensor_copy)(
                                eb, e_tiles[n][:, m * P:(m + 1) * P]
                            )
                            nc.tensor.transpose(
                                e_tiles[m][:, n * P:(n + 1) * P], eb, ident_f32
                            )

                    # attT_m = exp(E_m - shift[free]) (E symmetric: stored
                    # tiles double as E^T tiles), in two pieces: the strip
                    # region [m*P:] fires the moment strip m stops (MM2's
                    # m=3-first order only ever needs these early), the
                    # mirror region [:m*P] follows once the mirrors land.
                    att_t = [pool_att.tile([P, C], BF16, tag=f"attT{m}",
                                           name=f"attT{m}")
                             for m in range(IC)]
                    att8 = pool_att.tile([P, IC, C], FP8, tag="att8")

                    def emit_att_piece(m, c0, c1):
                        tmp = pool_scr.tile([P, C], F32, tag="tmp", name="tmp")
                        nc.vector.tensor_tensor(
                            tmp[:, c0:c1], e_tiles[m][:, c0:c1],
                            negmb[:, c0:c1], mybir.AluOpType.add,
                        )
                        nc.scalar.activation(
                            att_t[m][:, c0:c1], tmp[:, c0:c1],
                            mybir.ActivationFunctionType.Exp,
                        )
                        # fp8 copy for the DoubleRow pair terms of MM2; the
                        # high columns go first: the m=3/2 pair sweeps need
                        # them earliest
                        if c1 == C and c1 - c0 > C - 2 * P:
                            nc.gpsimd.tensor_copy(
                                att8[:, m, C - 2 * P:], att_t[m][:, C - 2 * P:]
                            )
                            nc.gpsimd.tensor_copy(
                                att8[:, m, c0:C - 2 * P],
                                att_t[m][:, c0:C - 2 * P],
                            )
                        else:
                            nc.gpsimd.tensor_copy(
                                att8[:, m, c0:c1], att_t[m][:, c0:c1]
                            )

                    for m in range(IC - 1, -1, -1):
                        emit_strip(cb, m, stop=True)
                        # wide strip-region pieces emit their high columns
                        # first: the fp8 pair sweeps and row sums need them
                        if C - m * P > C - 2 * P:
                            emit_att_piece(m, C - 2 * P, C)
                            emit_att_piece(m, m * P, C - 2 * P)
                        else:
                            emit_att_piece(m, m * P, C)
                        if m < IC - 1:
                            emit_mirrors(m)
                    for m in range(IC - 1, 0, -1):
                        emit_att_piece(m, 0, m * P)

                    # keep the PE warm through the start of the softmax
                    # bubble: harmless self-overwriting matmuls on the E0
                    # bank, after all its readers (the m=0 k-outer sweeps
                    # below take over as soon as attT_0 lands).
                    for _ in range(8):
                        nc.tensor.matmul(
                            e_tiles[0][:, :P], lhsT=qt[:, 0, :P],
                            rhs=qt[:, 0, :P],
                            start=True, stop=True, skip_group_check=True,
                        )

                # pe_/pt closed: the E banks release right after the tmp
                # adds, so MM2's PSUM buffers below never wait on them.
                gs = {}

                def emit_y(m, h, op, split=False):
                    # y = op * (gamma/s) + x via ACT-scale + bf16 DVE add:
                    # the PSUM bank frees at the end of the ACT copy, so MM2
                    # stays PE-paced. The very last chunk is emitted in two
                    # halves so its drain pipeline is half as deep.
                    n0 = h * YCHUNK
                    pieces = ((0, YCHUNK // 2), (YCHUNK // 2, YCHUNK)) \
                        if split else ((0, YCHUNK),)
                    for a, b in pieces:
                        yt = pool_y.tile([P, YCHUNK], BF16, tag="yt")
                        ts = pool_y.tile([P, YCHUNK], BF16, tag="ts")
                        nc.scalar.activation(
                            ts[:, :b - a], op[:, a:b],
                            mybir.ActivationFunctionType.Copy,
                            scale=gs[m],
                        )
                        nc.vector.tensor_tensor(
                            yt[:, :b - a], ts[:, :b - a],
                            xb_tiles[m][:, n0 + a:n0 + b],
                            mybir.AluOpType.add,
                        )
                        nc.sync.dma_start(
                            y[m * P:(m + 1) * P, n0 + a:n0 + b], yt[:, :b - a]
                        )

                # off-diagonal attention terms are ~exp(-large): the two
                # k-blocks away from the diagonal go through one fp8e4
                # DoubleRow matmul (half the PE rows), the diagonal block and
                # its neighbour stay bf16.
                PAIR_OF = {0: 2, 1: 2, 2: 0, 3: 0}   # first k of the fp8 pair
                SINGLE_OF = {0: 1, 1: 0, 2: 3, 3: 2}

                def emit_terms(m, h, op, order):
                    n0 = h * YCHUNK
                    s = SINGLE_OF[m]
                    pk = PAIR_OF[m]
                    for q in range(YCHUNK // C):
                        c0, c1 = n0 + q * C, n0 + (q + 1) * C
                        for t, term in enumerate(order):
                            if term == "diag":
                                lhs, rhs, pm = (att_t[m][:, m * P:(m + 1) * P],
                                                xb_tiles[m][:, c0:c1], None)
                            elif term == "single":
                                lhs, rhs, pm = (att_t[s][:, m * P:(m + 1) * P],
                                                xb_tiles[s][:, c0:c1], None)
                            else:
                                lhs, rhs, pm = (
                                    att8[:, pk:pk + 2, m * P:(m + 1) * P],
                                    xb8[:, pk:pk + 2, c0:c1],
                                    mybir.MatmulPerfMode.DoubleRow,
                                )
                            nc.tensor.matmul(
                                op[:, q * C:(q + 1) * C],
                                lhsT=lhs, rhs=rhs, perf_mode=pm,
                                start=(t == 0), stop=(t == len(order) - 1),
                            )

                def emit_chunk(m, h, po, split=False):
                    op = po.tile([P, YCHUNK], F32, tag="O")
                    emit_terms(m, h, op, ("diag", "single", "pair"))
                    emit_y(m, h, op, split=split)

                # --- MM2 + fused scale/residual + store (bf16) ---
                with (
                    tc.tile_pool(name="po", bufs=POB,
                                 space="PSUM") as po,
                    tc.tile_pool(name="psv", bufs=1, space="PSUM") as psv,
                ):
                    # m = 3, first KOUT chunks term-outer (see module
                    # docstring): the m=3 column block of every attT_k sits
                    # in the strip (non-mirror) region, so these sweeps only
                    # wait on the early exp pieces; sweep order follows
                    # operand readiness (single -> diag -> fp8 pair).
                    M0 = IC - 1
                    ops0 = [po.tile([P, YCHUNK], F32, tag="O", name=f"O3_{h}")
                            for h in range(KOUT)]
                    sv_col = psv.tile([P, IC], F32, tag="sv")
                    for t, term in enumerate(("diag", "single", "pair")):
                        for h in range(KOUT):
                            n0 = h * YCHUNK
                            s = SINGLE_OF[M0]
                            pk = PAIR_OF[M0]
                            for q in range(YCHUNK // C):
                                c0, c1 = n0 + q * C, n0 + (q + 1) * C
                                if term == "diag":
                                    lhs, rhs, pm = (
                                        att_t[M0][:, M0 * P:(M0 + 1) * P],
                                        xb_tiles[M0][:, c0:c1], None)
                                elif term == "single":
                                    lhs, rhs, pm = (
                                        att_t[s][:, M0 * P:(M0 + 1) * P],
                                        xb_tiles[s][:, c0:c1], None)
                                else:
                                    lhs, rhs, pm = (
                                        att8[:, pk:pk + 2,
                                             M0 * P:(M0 + 1) * P],
                                        xb8[:, pk:pk + 2, c0:c1],
                                        mybir.MatmulPerfMode.DoubleRow,
                                    )
                                nc.tensor.matmul(
                                    ops0[h][:, q * C:(q + 1) * C],
                                    lhsT=lhs, rhs=rhs, perf_mode=pm,
                                    start=(t == 0), stop=(t == 2),
                                )
                    # softmax row sums s_c = column sums of attT (symmetry),
                    # accumulated straight into per-partition columns:
                    # out[i, 0] = sum_p attT_k[p, m*P+i]; m-outer so only one
                    # accumulation group is pending per bank at a time, m=3
                    # first since y(3, *) needs gs[3] first.
                    for m in range(IC - 1, -1, -1):
                        for k in range(IC):
                            nc.tensor.matmul(
                                sv_col[:, m:m + 1],
                                lhsT=att_t[k][:, m * P:(m + 1) * P],
                                rhs=ones_col,
                                start=(k == 0), stop=(k == IC - 1),
                            )
                        iv = small.tile([P, 1], F32, tag=f"inv{m}", name=f"inv{m}")
                        gsm = small.tile([P, 1], F32, tag=f"gs{m}", name=f"gs{m}")
                        nc.vector.reciprocal(iv, sv_col[:, m:m + 1])
                        nc.vector.tensor_tensor(
                            gsm, iv, gvec, mybir.AluOpType.mult
                        )
                        gs[m] = gsm
                    for h in range(KOUT):
                        emit_y(M0, h, ops0[h])
                    for h in range(KOUT, NH):
                        emit_chunk(M0, h, po)
                    for m in range(IC - 2, -1, -1):
                        for h in range(NH):
                            emit_chunk(m, h, po,
                                       split=(m == 0 and h == NH - 1))


def build_kernel(reps: int = 1, loop_iters: int = 0):
    nc = bacc.Bacc("TRN2", target_bir_lowering=False)
    x = nc.dram_tensor("x", [C, N], F32, kind="ExternalInput")
    x2 = nc.dram_tensor("x2", [C, N], F32, kind="ExternalInput")
    gamma = nc.dram_tensor("gamma", [1, 1], F32, kind="ExternalInput")
    y = nc.dram_tensor("y", [C, N], BF16, kind="ExternalOutput")

    with TileContext(nc) as tc:
        if loop_iters:
            engs = [mybir.EngineType.PE, mybir.EngineType.DVE,
                    mybir.EngineType.Activation, mybir.EngineType.SP,
                    mybir.EngineType.Pool]
            with tc.For_i(0, loop_iters, 1, hint_engines=engs):
                _emit_core(nc, tc, x, x2, gamma, y)
        else:
            for _ in range(reps):
                _emit_core(nc, tc, x, x2, gamma, y)

    nc.finalize()
    return nc


_NC_CACHE = None


def _get_nc():
    global _NC_CACHE
    if _NC_CACHE is None:
        _NC_CACHE = build_kernel()
    return _NC_CACHE


def kernel(x: np.ndarray, x2: np.ndarray, gamma: np.ndarray) -> np.ndarray:
    from concourse.bass_utils import run_bass_kernel_spmd

    nc = _get_nc()
    xf = np.ascontiguousarray(np.asarray(x, dtype=np.float32)).reshape(B, C, N)
    x2f = np.ascontiguousarray(np.asarray(x2, dtype=np.float32)).reshape(B, C, N)
    gf = np.asarray(gamma, dtype=np.float32).reshape(1, 1)
    in_maps = [{"x": xf[b], "x2": x2f[b], "gamma": gf} for b in range(B)]
    res = run_bass_kernel_spmd(nc, in_maps, core_ids=list(range(B)))
    out = np.stack(
        [np.asarray(res.results[b]["y"], dtype=np.float32) for b in range(B)],
        axis=0,
    )
    return out.reshape(x.shape)


if __name__ == "__main__":
    rng = np.random.default_rng(0)
    x = rng.standard_normal((B, C, 64, 64), dtype=np.float32)
    x2 = rng.standard_normal((B, C, 64, 64), dtype=np.float32)
    gamma = np.zeros((1,), dtype=np.float32)
    out = kernel(x=x, x2=x2, gamma=gamma)
    print("shape:", out.shape, "dtype:", out.dtype)
    print("max |out - x| (gamma=0 => should be ~bf16 eps):", np.abs(out - x).max())



# revision 8
# speedup vs baseline: 1.2228x; 1.2228x over previous
"""Trainium2 Bass kernel for CAM (channel attention module).

reference:
    q = k = x2.reshape(B, C, N); v = x.reshape(B, C, N)   # B=8, C=512, N=4096
    energy = q @ q^T                # [B, C, C]
    att = softmax(energy, axis=-1)
    out = att @ v
    y = gamma * out + x

Sharding: data-parallel over batch, one batch element per NeuronCore (8 cores).
Each core computes its own [C, N] slice end to end; no collectives.

Per-core dataflow (C=512, N=4096, P=128):
  1. stream x2 in 512-wide column chunks (one fused SWDGE cast-DMA
     f32 -> bf16 per chunk covering all 4 c-tiles; the first chunk is two
     256-wide halves so the PE starts sooner). PE-transpose 128x128 blocks
     -> qT bf16 [n-part, c-free]; MM1 for chunk i-1 is emitted right after
     chunk i's transposes, so the PE sees one continuous stream.
  2. MM1 (bf16): E [C, C] in PSUM f32, symmetric: each row-tile
     accumulates one strip [m*P:] (diag included; one PSUM group per
     bank); lower blocks are PE transposes of the mirrors. Final-chunk
     strips are emitted m=3 -> m=0 so the narrow strips stop first.
  3. softmax shift = Gram diagonal ||q_c||^2 (valid shift by symmetry;
     softmax is shift-invariant): partial sums of squares accumulate on
     ACT (Square+accum) and DVE (mult+reduce) during the otherwise-idle
     load phase, tree-add on gpsimd, then one rank-1 PE matmul broadcasts
     the negated row to [128, C] well before the strips stop.
  4. attT_m = exp(E_m - shift[free]) in two pieces per row-tile: the strip
     region [m*P:] fires the moment strip m stops, the mirror region
     [:m*P] once the mirrors land. gpsimd mirrors each piece into fp8.
  5. MM2 (all bf16): x streams in as four [128, 4, 1024] n-chunks behind
     the x2 stream (total HBM traffic 20MB/core: x2 8 + x 8 + y 4; the
     old fp8 re-read of x cost 8MB more). Each output chunk (m, h)
     depends on x chunk h only, so outputs complete progressively and
     the y stores interleave with the tail of the load stream -- the DMA
     rings stay saturated end to end. m=3 goes first within each chunk
     because its att columns avoid the mirror region entirely. Softmax
     row sums ride along as PE column-sums of attT (symmetry).
  6. y = (out * gamma/s_c) + x_bf16 via ACT-scale (per-partition gs AP)
     + bf16 DVE add; the PSUM bank frees at the ACT copy so MM2 stays
     PE-paced. Stored as bf16 (host upcasts; ~3e-3 rel err, 2e-2 gate).

Known hardware pitfalls encoded here: GPSIMD must not touch PSUM;
tensor_tensor_reduce crashes the exec unit (use mult + tensor_reduce);
only gpsimd DMAs can cast; PSUM allows one open accumulation group per
bank; list-scheduler hoists ready DMAs past program order (pin with
sliver dependencies).
"""

import numpy as np

import concourse.bass as bass
import concourse.mybir as mybir
from concourse import bacc
from concourse.tile import TileContext
from concourse.masks import make_identity

P = 128
C = 512
N = 4096
B = 8
IC = C // P          # 4 c-tiles
JN = N // P          # 32 n-tiles
F32 = mybir.dt.float32
BF16 = mybir.dt.bfloat16
FP8 = mybir.dt.float8e4

QCHUNK = 512         # x2 load chunk width (free dim)
# first chunk split in two so the PE pipeline starts ~2us earlier
CHUNK_W = [QCHUNK // 2, QCHUNK // 2] + [QCHUNK] * (N // QCHUNK - 1)
CHUNK_J0 = [sum(CHUNK_W[:i]) // P for i in range(len(CHUNK_W))]
NCB = len(CHUNK_W)   # chunks
YCHUNK = 1024        # x load / y store chunk width (free dim)
NH = N // YCHUNK     # x/y chunks
POB = 3              # MM2 PSUM output buffers ([P,1024] f32 = 2 banks each)


def _emit_core(nc, tc, x, x2, gamma, y):
    with (
            tc.tile_pool(name="small", bufs=1) as small,
            tc.tile_pool(name="vpool", bufs=NH) as pool_v,
            tc.tile_pool(name="att", bufs=2) as pool_att,
            tc.tile_pool(name="scr", bufs=3) as pool_scr,
            tc.tile_pool(name="ypool", bufs=4) as pool_y,
            tc.tile_pool(name="qn_", bufs=NCB - 2) as pool_qn,
            tc.tile_pool(name="qt_", bufs=1) as pool_qt,
        ):
            # chunk 0's cast-DMA desc-gen goes first on the Pool engine;
            # the transpose identity (only constant the x2 pipeline needs)
            # builds while that DMA is in flight
            x2v = x2.rearrange("(i p) n -> p i n", p=P)
            qn_chunks = []

            def emit_qn_dma(cb):
                # one fused cast-DMA per chunk for all 4 c-tiles
                # (SWDGE fixed desc-gen cost is per-instruction)
                w0, wd = CHUNK_J0[cb] * P, CHUNK_W[cb]
                qn = pool_qn.tile([P, IC, QCHUNK], BF16, tag="qn")
                nc.gpsimd.dma_start(qn[:, :, :wd], x2v[:, :, w0:w0 + wd])
                qn_chunks.append(qn)

            emit_qn_dma(0)
            ident_bf = small.tile([P, P], BF16, tag="ident_bf")
            make_identity(nc, ident_bf)
            for cb in range(1, NCB):
                emit_qn_dma(cb)

            # --- remaining constants / tiny tensors ---
            ident_f32 = small.tile([P, P], F32, tag="ident_f32")
            make_identity(nc, ident_f32)
            ones_row = small.tile([1, P], F32, tag="ones_row")
            nc.any.memset(ones_row, 1.0)
            neg_ones = small.tile([1, P], F32, tag="neg_ones")
            nc.any.memset(neg_ones, -1.0)
            ones_col = small.tile([P, 1], BF16, tag="ones_col")
            nc.any.memset(ones_col, 1.0)
            # preload the Exp table off the critical path
            warm_act = small.tile([1, P], BF16, tag="warm_act")
            nc.scalar.activation(
                warm_act, ones_row, mybir.ActivationFunctionType.Exp
            )
            g_sb = small.tile([1, 1], F32, tag="g_sb")
            nc.scalar.dma_start(g_sb, gamma[:, :])
            gvec = small.tile([P, 1], F32, tag="gvec")
            with tc.tile_pool(name="pg", bufs=1, space="PSUM") as pg:
                gp = pg.tile([P, 1], F32, tag="gp")
                # gvec[p] = gamma for all p  (rank-1 broadcast via PE)
                nc.tensor.matmul(gp, lhsT=ones_row, rhs=g_sb, start=True, stop=True)
                nc.vector.tensor_copy(gvec, gp)

            xch = []             # x bf16 n-chunks [P, IC, YCHUNK]
            qt = pool_qt.tile([P, JN, P * IC], BF16, tag="qt")  # [128,32,512]

            # pb holds the tiny PSUM tensors whose lifetime crosses the
            # MM1 -> MM2 boundary (2 banks, rotating: shift row / shift
            # broadcast / row-sum row / row-sum columns)
            import contextlib
            with contextlib.nullcontext():
                with (
                    tc.tile_pool(name="pt", bufs=2, space="PSUM") as pt,
                    tc.tile_pool(name="pe_", bufs=4, space="PSUM") as pe_,
                ):
                    # E accumulators live across the whole streamed MM1
                    e_tiles = [pe_.tile([P, C], F32, tag="E", name=f"E{m}")
                               for m in range(IC)]

                    # E is symmetric: each row-tile accumulates one strip
                    # [m*P:] (diag included -- a PSUM bank allows only one
                    # open accumulation group); lower blocks are PE
                    # transposes of the mirrors, filled in as strips stop.
                    def emit_strip(cb, m, stop):
                        jpc = CHUNK_W[cb] // P
                        for jj in range(jpc):
                            j = CHUNK_J0[cb] + jj
                            nc.tensor.matmul(
                                e_tiles[m][:, m * P:],
                                lhsT=qt[:, j, m * P:(m + 1) * P],
                                rhs=qt[:, j, m * P:],
                                start=(j == 0),
                                stop=(stop and jj == jpc - 1),
                            )

                    def emit_mm1(cb):
                        for m in range(IC):
                            emit_strip(cb, m, stop=False)

                    ssq_parts = [[None] * NCB for _ in range(IC)]
                    negmb = small.tile([P, C], F32, tag="negmb")

                    def emit_x_loads():
                        # v (= x) cast-loads queue behind x2 on the same
                        # SWDGE FIFO (desc-gen before the gpsimd tree-adds),
                        # as [P, IC, YCHUNK] n-chunks so each MM2 output
                        # chunk depends on one x chunk only. Each DMA
                        # overwrites a sliver copied from a late x2 chunk
                        # so the scheduler cannot hoist it ahead of the
                        # stream.
                        xv = x.rearrange("(i p) n -> p i n", p=P)
                        for h in range(NH):
                            xc = pool_v.tile([P, IC, YCHUNK], BF16,
                                             tag="xch", name=f"xch{h}")
                            nc.gpsimd.tensor_copy(
                                xc[:, 0, :1], qn_chunks[NCB - 2][:, 0, :1]
                            )
                            nc.gpsimd.dma_start(
                                xc, xv[:, :, h * YCHUNK:(h + 1) * YCHUNK]
                            )
                            xch.append(xc)

                    def emit_shift_chain():
                        # ssq_i = ||q_c||^2 per c-tile: tree-add the square
                        # partials on gpsimd (idle after desc-gen; SBUF-only),
                        # then shift as a row [1, C], broadcast (negated, via
                        # the minus-ones lhsT) to [128, C] on PE. Emitted
                        # inside the load loop so the PE hits the transposes
                        # when their inputs are already long ready, well
                        # before MM1's final strips stop. The PSUM pool is
                        # scoped to the chain so its banks free immediately.
                        ssq = [None] * IC
                        for i in range(IC - 1, -1, -1):
                            acc = small.tile([P, 1], F32, tag=f"ssqa{i}",
                                             name=f"ssqa{i}")
                            nc.gpsimd.tensor_tensor(
                                acc, ssq_parts[i][0], ssq_parts[i][1],
                                mybir.AluOpType.add,
                            )
                            for cc in range(2, NCB):
                                nc.gpsimd.tensor_tensor(
                                    acc, acc, ssq_parts[i][cc],
                                    mybir.AluOpType.add,
                                )
                            ssq[i] = acc
                        # built per 128-column piece, m=3 first, so each
                        # piece's exp chain fires as soon as its strip stops
                        # instead of waiting for the full-row broadcast
                        with tc.tile_pool(name="pbx", bufs=1,
                                          space="PSUM") as pbx:
                            mrow_p = pbx.tile([1, C], F32, tag="rowbuf")
                            mrow_sb = small.tile([1, C], F32, tag="mrow_sb")
                            negmb_p = pbx.tile([P, C], F32, tag="bcbuf")
                            for m in range(IC - 1, -1, -1):
                                c0, c1 = m * P, (m + 1) * P
                                nc.tensor.transpose(
                                    mrow_p[:, c0:c1], ssq[m], ident_f32,
                                )
                                nc.vector.tensor_copy(
                                    mrow_sb[:, c0:c1], mrow_p[:, c0:c1]
                                )
                                nc.tensor.matmul(
                                    negmb_p[:, c0:c1], lhsT=neg_ones,
                                    rhs=mrow_sb[:, c0:c1],
                                    start=True, stop=True,
                                )
                                nc.scalar.copy(
                                    negmb[:, c0:c1], negmb_p[:, c0:c1]
                                )

                    for cb in range(NCB):
                        qn = qn_chunks[cb]
                        for jj in range(CHUNK_W[cb] // P):
                            j = CHUNK_J0[cb] + jj
                            ps = pt.tile([P, P * IC], BF16, tag="ps")
                            for i in range(IC):
                                nc.tensor.transpose(
                                    ps[:, i * P:(i + 1) * P],
                                    qn[:, i, jj * P:(jj + 1) * P],
                                    ident_bf,
                                )
                            nc.vector.tensor_copy(out=qt[:, j, :], in_=ps)
                        # partial sum-of-squares of q: the softmax shift is
                        # the Gram diagonal ||q_c||^2 (instead of a row max),
                        # computed on otherwise-idle ACT/DVE during the load
                        # phase so the shift broadcast is ready long before
                        # MM1 finishes
                        for i in range(IC):
                            pp = small.tile([P, 1], F32, tag=f"ssq{i}_{cb}",
                                            name=f"ssq{i}_{cb}")
                            sq = pool_scr.tile([P, QCHUNK], BF16, tag="sq",
                                               name="sq")
                            wd = CHUNK_W[cb]
                            if (cb * IC + i) % 3 != 1:
                                nc.scalar.activation(
                                    sq[:, :wd], qn[:, i, :wd],
                                    mybir.ActivationFunctionType.Square,
                                    accum_out=pp,
                                )
                            else:
                                # (tensor_tensor_reduce crashes the exec
                                # unit on TRN2 hardware; use mult + reduce)
                                nc.vector.tensor_tensor(
                                    sq[:, :wd], qn[:, i, :wd], qn[:, i, :wd],
                                    mybir.AluOpType.mult,
                                )
                                nc.vector.tensor_reduce(
                                    pp, sq[:, :wd], mybir.AxisListType.X,
                                    mybir.AluOpType.add,
                                )
                            ssq_parts[i][cb] = pp
                        if cb == NCB - 1:
                            emit_x_loads()
                            emit_shift_chain()
                        if cb > 0:
                            emit_mm1(cb - 1)

                    # --- final chunk: staggered strips -> mirrors -> exp ---
                    cb = NCB - 1

                    def emit_mirrors(n):
                        # lower blocks of rows m > n from strip n's mirrors:
                        # E_m[:, n] = E_n[:, m]^T (copies on ACT: it may read
                        # PSUM, keeping DVE free for the tmp adds)
                        for m in range(n + 1, IC):
                            eb = pool_scr.tile([P, P], F32, tag="eb", name="eb")
                            ceng = nc.scalar if (m + n) % 2 == 0 else nc.vector
                            (ceng.copy if ceng is nc.scalar
                             else ceng.tensor_copy)(
                                eb, e_tiles[n][:, m * P:(m + 1) * P]
                            )
                            nc.tensor.transpose(
                                e_tiles[m][:, n * P:(n + 1) * P], eb, ident_f32
                            )

                    # attT_m = exp(E_m - shift[free]) (E symmetric: stored
                    # tiles double as E^T tiles), in two pieces: the strip
                    # region [m*P:] fires the moment strip m stops (MM2's
                    # m=3-first order only ever needs these early), the
                    # mirror region [:m*P] follows once the mirrors land.
                    att_t = [pool_att.tile([P, C], BF16, tag=f"attT{m}",
                                           name=f"attT{m}")
                             for m in range(IC)]

                    def emit_att_piece(m, c0, c1):
                        tmp = pool_scr.tile([P, C], F32, tag="tmp", name="tmp")
                        nc.vector.tensor_tensor(
                            tmp[:, c0:c1], e_tiles[m][:, c0:c1],
                            negmb[:, c0:c1], mybir.AluOpType.add,
                        )
                        nc.scalar.activation(
                            att_t[m][:, c0:c1], tmp[:, c0:c1],
                            mybir.ActivationFunctionType.Exp,
                        )

                    for m in range(IC - 1, -1, -1):
                        emit_strip(cb, m, stop=True)
                        # wide strip-region pieces emit their high columns
                        # first: the fp8 pair sweeps and row sums need them
                        if C - m * P > C - 2 * P:
                            emit_att_piece(m, C - 2 * P, C)
                            emit_att_piece(m, m * P, C - 2 * P)
                        else:
                            emit_att_piece(m, m * P, C)
                        if m < IC - 1:
                            emit_mirrors(m)
                    for m in range(IC - 1, 0, -1):
                        emit_att_piece(m, 0, m * P)

                    # keep the PE warm through the start of the softmax
                    # bubble: harmless self-overwriting matmuls on the E0
                    # bank, after all its readers (the m=0 k-outer sweeps
                    # below take over as soon as attT_0 lands).
                    for _ in range(8):
                        nc.tensor.matmul(
                            e_tiles[0][:, :P], lhsT=qt[:, 0, :P],
                            rhs=qt[:, 0, :P],
                            start=True, stop=True, skip_group_check=True,
                        )

                # pe_/pt closed: the E banks release right after the tmp
                # adds, so MM2's PSUM buffers below never wait on them.
                gs = {}

                def emit_y(m, h, op, split=False):
                    # y = op * (gamma/s) + x via ACT-scale + bf16 DVE add:
                    # the PSUM bank frees at the end of the ACT copy, so MM2
                    # stays PE-paced. The very last chunk is emitted in two
                    # halves so its drain pipeline is half as deep.
                    n0 = h * YCHUNK
                    pieces = ((0, YCHUNK // 2), (YCHUNK // 2, YCHUNK)) \
                        if split else ((0, YCHUNK),)
                    for a, b in pieces:
                        yt = pool_y.tile([P, YCHUNK], BF16, tag="yt")
                        ts = pool_y.tile([P, YCHUNK], BF16, tag="ts")
                        nc.scalar.activation(
                            ts[:, :b - a], op[:, a:b],
                            mybir.ActivationFunctionType.Copy,
                            scale=gs[m],
                        )
                        nc.vector.tensor_tensor(
                            yt[:, :b - a], ts[:, :b - a],
                            xch[h][:, m, a:b],
                            mybir.AluOpType.add,
                        )
                        nc.sync.dma_start(
                            y[m * P:(m + 1) * P, n0 + a:n0 + b], yt[:, :b - a]
                        )

                def emit_sv(m):
                    # softmax row sums s_c = column sums of attT (symmetry),
                    # accumulated straight into per-partition columns:
                    # out[i, 0] = sum_p attT_k[p, m*P+i]; one accumulation
                    # group pending on the sv bank at a time.
                    for k in range(IC):
                        nc.tensor.matmul(
                            sv_col[:, m:m + 1],
                            lhsT=att_t[k][:, m * P:(m + 1) * P],
                            rhs=ones_col,
                            start=(k == 0), stop=(k == IC - 1),
                        )
                    iv = small.tile([P, 1], F32, tag=f"inv{m}", name=f"inv{m}")
                    gsm = small.tile([P, 1], F32, tag=f"gs{m}", name=f"gs{m}")
                    nc.vector.reciprocal(iv, sv_col[:, m:m + 1])
                    nc.vector.tensor_tensor(
                        gsm, iv, gvec, mybir.AluOpType.mult
                    )
                    gs[m] = gsm

                def emit_chunk(m, h, po, split=False):
                    # all-bf16 k-sweep: op(m, h) = sum_k attT_k[m-cols]^T @
                    # x_chunk_h[k]; q-outer so one accumulation group per
                    # PSUM bank is pending at a time.
                    op = po.tile([P, YCHUNK], F32, tag="O")
                    for q in range(YCHUNK // C):
                        for k in range(IC):
                            nc.tensor.matmul(
                                op[:, q * C:(q + 1) * C],
                                lhsT=att_t[k][:, m * P:(m + 1) * P],
                                rhs=xch[h][:, k, q * C:(q + 1) * C],
                                start=(k == 0), stop=(k == IC - 1),
                            )
                    emit_y(m, h, op, split=split)

                # --- MM2 + fused scale/residual + store (bf16) ---
                # h-outer so chunk (m, h) only waits on x chunk h; m=3 first
                # within each chunk (its att columns sit entirely in the
                # strip region, so h=0/m=3 starts right at the first exp;
                # the m<3 row sums need the mirrors, so they're emitted
                # after the first sweep).
                with (
                    tc.tile_pool(name="po", bufs=POB,
                                 space="PSUM") as po,
                    tc.tile_pool(name="psv", bufs=1, space="PSUM") as psv,
                ):
                    sv_col = psv.tile([P, IC], F32, tag="sv")
                    emit_sv(IC - 1)
                    emit_chunk(IC - 1, 0, po)
                    for m in range(IC - 2, -1, -1):
                        emit_sv(m)
                    for m in range(IC - 2, -1, -1):
                        emit_chunk(m, 0, po)
                    for h in range(1, NH):
                        for m in range(IC - 1, -1, -1):
                            emit_chunk(m, h, po,
                                       split=(m == 0 and h == NH - 1))


def build_kernel(reps: int = 1, loop_iters: int = 0):
    nc = bacc.Bacc("TRN2", target_bir_lowering=False)
    x = nc.dram_tensor("x", [C, N], F32, kind="ExternalInput")
    x2 = nc.dram_tensor("x2", [C, N], F32, kind="ExternalInput")
    gamma = nc.dram_tensor("gamma", [1, 1], F32, kind="ExternalInput")
    y = nc.dram_tensor("y", [C, N], BF16, kind="ExternalOutput")

    with TileContext(nc) as tc:
        if loop_iters:
            engs = [mybir.EngineType.PE, mybir.EngineType.DVE,
                    mybir.EngineType.Activation, mybir.EngineType.SP,
                    mybir.EngineType.Pool]
            with tc.For_i(0, loop_iters, 1, hint_engines=engs):
                _emit_core(nc, tc, x, x2, gamma, y)
        else:
            for _ in range(reps):
                _emit_core(nc, tc, x, x2, gamma, y)

    nc.finalize()
    return nc


_NC_CACHE = None


def _get_nc():
    global _NC_CACHE
    if _NC_CACHE is None:
        _NC_CACHE = build_kernel()
    return _NC_CACHE


def kernel(x: np.ndarray, x2: np.ndarray, gamma: np.ndarray) -> np.ndarray:
    from concourse.bass_utils import run_bass_kernel_spmd

    nc = _get_nc()
    xf = np.ascontiguousarray(np.asarray(x, dtype=np.float32)).reshape(B, C, N)
    x2f = np.ascontiguousarray(np.asarray(x2, dtype=np.float32)).reshape(B, C, N)
    gf = np.asarray(gamma, dtype=np.float32).reshape(1, 1)
    in_maps = [{"x": xf[b], "x2": x2f[b], "gamma": gf} for b in range(B)]
    res = run_bass_kernel_spmd(nc, in_maps, core_ids=list(range(B)))
    out = np.stack(
        [np.asarray(res.results[b]["y"], dtype=np.float32) for b in range(B)],
        axis=0,
    )
    return out.reshape(x.shape)


if __name__ == "__main__":
    rng = np.random.default_rng(0)
    x = rng.standard_normal((B, C, 64, 64), dtype=np.float32)
    x2 = rng.standard_normal((B, C, 64, 64), dtype=np.float32)
    gamma = np.zeros((1,), dtype=np.float32)
    out = kernel(x=x, x2=x2, gamma=gamma)
    print("shape:", out.shape, "dtype:", out.dtype)
    print("max |out - x| (gamma=0 => should be ~bf16 eps):", np.abs(out - x).max())

